# revision 1
# baseline (speedup 1.0000x reference)
"""GATv2 actor layer (nn_GATv2Actor) on 8 TRN2 NeuronCores via Bass/Tile.

Self-contained: kernel(**inputs) takes the full (unsharded) inputs of
reference.setup_inputs() and returns the full [50000, 4] float32 output.

Distribution strategy (edge-parallel by destination-node range):
  - node n is owned by core n // 6250; each core handles all edges whose
    destination lies in its range (plus its self-loops), so the segment
    softmax and the scatter-add are fully core-local and the final
    output rows are disjoint (host just concatenates - no collective).
  - per-node fp16 tables are computed on device (phase A):
      asv[n]  = [h@pair_W_src | h@value_W]  (gathered by edge src)
      adst[n] = h@pair_W_dst + pair_b       (gathered by edge dst)
  - phase B: edges sorted by (dst, src), tiled 128/partition-dim tile;
    dma_gather fetches table rows; dense fp16 edge math (leaky-relu,
    attention logits, exp); a one-hot(dst-within-128-node-block) matmul
    on the TensorEngine accumulates [ex-weighted values | ex] into a
    per-block PSUM - this implements both segment_sum scatters at once.
    Softmax max-subtraction is skipped (logits are in [-1, 1.2]; softmax
    is shift-invariant so the result only differs by fp rounding).
  - phase C: per block, agg = U/denom, output MLP + phase softmax, DMA.

SPMD uniformity: one program runs on all 8 cores; per-(block,stream)
tile counts are padded to the max over cores. int16 gather indices
limit tables to 32767 rows, so edges are split into two streams by
src < 32768 gathering from two base offsets of the asv table.
"""
import math
import sys

import numpy as np

sys.path.insert(0, "/opt/trn_rl_repo")

import concourse.bass as bass  # noqa: E402
import concourse.tile as tile  # noqa: E402
from concourse import bacc, mybir  # noqa: E402
from concourse.bass_utils import run_bass_kernel_spmd  # noqa: E402

FP16 = mybir.dt.float16
F32 = mybir.dt.float32
I16 = mybir.dt.int16
AT = mybir.AluOpType
ACTF = mybir.ActivationFunctionType

F = 128      # feature dim
H = 2        # heads
D = 64       # head dim
P_OUT = 4    # phases
N_CORES = 8


def prep(h_int, edge_index, pair_W, pair_b, attn_w, value_W, out_W, out_b,
         phase_W, phase_b, n_cores=N_CORES, G=24, split=32768, A_CH=8):
    """Host-side index preprocessing + input packing. Returns (meta, in_maps)."""
    h = np.asarray(h_int, np.float32)
    ei = np.asarray(edge_index)
    pair_W = np.asarray(pair_W, np.float32)
    pair_b = np.asarray(pair_b, np.float32)
    attn_w = np.asarray(attn_w, np.float32)
    value_W = np.asarray(value_W, np.float32)
    out_W = np.asarray(out_W, np.float32)
    out_b = np.asarray(out_b, np.float32)
    phase_W = np.asarray(phase_W, np.float32)
    phase_b = np.asarray(phase_b, np.float32)
    N = h.shape[0]
    assert N % n_cores == 0
    NPC = N // n_cores
    NBLK = (NPC + 127) // 128
    NPAD = ((N + 127) // 128) * 128
    assert NPAD - split < 32768 and split < 32768 + 1

    src = np.concatenate([ei[0], np.arange(N)]).astype(np.int64)
    dst = np.concatenate([ei[1], np.arange(N)]).astype(np.int64)
    core = dst // NPC

    percore = []
    counts = np.zeros((n_cores, 2, NBLK), np.int64)
    for c in range(n_cores):
        m = core == c
        es = src[m]
        ed = dst[m] - c * NPC
        o = np.lexsort((es, ed))
        es, ed = es[o], ed[o]
        lo = es < split
        percore.append((es, ed, lo))
        for si in range(2):
            msk = lo if si == 0 else ~lo
            counts[c, si] = np.bincount(ed[msk] // 128, minlength=NBLK)
    T = np.ceil(counts.max(axis=0) / 128.0).astype(np.int64)  # [2, NBLK]
    tiles = T.sum(axis=1)
    L = tiles * 128
    base_tile = np.zeros((2, NBLK + 1), np.int64)
    base_tile[:, 1:] = np.cumsum(T, axis=1)

    f16 = np.float16
    W_asv = np.concatenate([pair_W[0, :F], pair_W[1, :F],
                            value_W[0], value_W[1]], axis=1).astype(f16)
    W_dst = np.concatenate([pair_W[0, F:], pair_W[1, F:]], axis=1).astype(f16)
    bias_bc = np.broadcast_to(np.asarray(pair_b, np.float32).reshape(-1),
                              (128, F)).copy()
    attn_bc = np.broadcast_to(np.asarray(attn_w, f16).reshape(-1), (128, F)).copy()
    iota_bc = np.broadcast_to(np.arange(128, dtype=f16), (128, 128)).copy()
    ident = np.eye(128, dtype=np.float32)
    out_Wt = np.asarray(out_W, f16)
    out_b_c = np.asarray(out_b, np.float32).reshape(128, 1).copy()
    phase_Wt = np.asarray(phase_W, f16)
    phase_b_bc = np.broadcast_to(np.asarray(phase_b, np.float32), (128, P_OUT)).copy()

    hp = np.zeros((NPAD, F), np.float32)
    hp[:N] = h
    hT16 = np.ascontiguousarray(hp.T.astype(f16))

    shared = dict(hT16=hT16, W_asv=W_asv, W_dst=W_dst, bias_bc=bias_bc,
                  attn_bc=attn_bc, iota_bc=iota_bc, ident=ident, out_Wt=out_Wt,
                  out_b=out_b_c, phase_Wt=phase_Wt, phase_b_bc=phase_b_bc)

    in_maps = []
    for c in range(n_cores):
        es, ed, lo = percore[c]
        m = {}
        for si in range(2):
            msk = lo if si == 0 else ~lo
            es_s, ed_s = es[msk], ed[msk]
            gidx = np.zeros(L[si], np.int16)
            dloc = np.zeros(L[si], np.int16)
            drel = np.full(L[si], -1.0, np.float16)
            starts = np.searchsorted(ed_s, np.arange(NBLK + 1) * 128)
            for j in range(NBLK):
                seg = slice(starts[j], starts[j + 1])
                n = starts[j + 1] - starts[j]
                b0 = base_tile[si, j] * 128
                gidx[b0:b0 + n] = (es_s[seg] - (split if si else 0)).astype(np.int16)
                dloc[b0:b0 + n] = ed_s[seg].astype(np.int16)
                drel[b0:b0 + n] = (ed_s[seg] - j * 128).astype(np.float16)
            m[f"gw{si}"] = np.tile(gidx.reshape(-1, 16).T, (8, 1)).copy()
            m[f"dw{si}"] = np.tile(dloc.reshape(-1, 16).T, (8, 1)).copy()
            m[f"drel{si}"] = np.ascontiguousarray(drel.reshape(-1, 128).T)
        hl = np.zeros((NBLK * 128, F), np.float32)
        hl[:NPC] = h[c * NPC:(c + 1) * NPC]
        m["hlocT16"] = np.ascontiguousarray(hl.T.astype(f16))
        m.update(shared)
        in_maps.append(m)

    meta = dict(N=N, NPC=NPC, NBLK=NBLK, NPAD=NPAD, split=split, G=G,
                A_CH=A_CH, T=T, tiles=tiles, L=L, base_tile=base_tile,
                n_cores=n_cores)
    return meta, in_maps


def build(meta):
    NPC, NBLK, NPAD = meta["NPC"], meta["NBLK"], meta["NPAD"]
    split, G, A_CH = meta["split"], meta["G"], meta["A_CH"]
    T, tiles, L = meta["T"], meta["tiles"], meta["L"]
    base_tile = meta["base_tile"]
    last_rows = NPC - (NBLK - 1) * 128

    nc = bacc.Bacc(None, target_bir_lowering=False, debug=False)

    hT_d = nc.dram_tensor("hT16", [128, NPAD], FP16, kind="ExternalInput")
    hloc_d = nc.dram_tensor("hlocT16", [128, NBLK * 128], FP16, kind="ExternalInput")
    gw_d = [nc.dram_tensor(f"gw{s}", [128, int(L[s]) // 16], I16,
                           kind="ExternalInput") for s in range(2)]
    dw_d = [nc.dram_tensor(f"dw{s}", [128, int(L[s]) // 16], I16,
                           kind="ExternalInput") for s in range(2)]
    drel_d = [nc.dram_tensor(f"drel{s}", [128, int(tiles[s])], FP16,
                             kind="ExternalInput") for s in range(2)]
    Wasv_d = nc.dram_tensor("W_asv", [128, 256], FP16, kind="ExternalInput")
    Wdst_d = nc.dram_tensor("W_dst", [128, 128], FP16, kind="ExternalInput")
    bias_d = nc.dram_tensor("bias_bc", [128, 128], F32, kind="ExternalInput")
    attn_d = nc.dram_tensor("attn_bc", [128, 128], FP16, kind="ExternalInput")
    iota_d = nc.dram_tensor("iota_bc", [128, 128], FP16, kind="ExternalInput")
    ident_d = nc.dram_tensor("ident", [128, 128], F32, kind="ExternalInput")
    outW_d = nc.dram_tensor("out_Wt", [128, 128], FP16, kind="ExternalInput")
    outb_d = nc.dram_tensor("out_b", [128, 1], F32, kind="ExternalInput")
    phW_d = nc.dram_tensor("phase_Wt", [128, P_OUT], FP16, kind="ExternalInput")
    phb_d = nc.dram_tensor("phase_b_bc", [128, P_OUT], F32, kind="ExternalInput")

    asv_d = nc.dram_tensor("asv_tab", [NPAD, 256], FP16)
    adst_d = nc.dram_tensor("adst_tab", [NBLK * 128, 128], FP16)
    out_d = nc.dram_tensor("out", [NPC, P_OUT], F32, kind="ExternalOutput")

    with tile.TileContext(nc) as tc:
        with tc.tile_pool(name="consts", bufs=1) as pc:
            def cload(name, dram, shape, dtype):
                t = pc.tile(shape, dtype, tag=name)
                nc.sync.dma_start(t[:], dram[:])
                return t
            Wasv = cload("Wasv", Wasv_d, [128, 256], FP16)
            Wdst = cload("Wdst", Wdst_d, [128, 128], FP16)
            bias = cload("bias", bias_d, [128, 128], F32)
            attn = cload("attn", attn_d, [128, 128], FP16)
            iota = cload("iota", iota_d, [128, 128], FP16)
            ident_f32 = cload("ident", ident_d, [128, 128], F32)
            outW = cload("outW", outW_d, [128, 128], FP16)
            outb = cload("outb", outb_d, [128, 1], F32)
            phW = cload("phW", phW_d, [128, P_OUT], FP16)
            phb = cload("phb", phb_d, [128, P_OUT], F32)
            gw = [cload(f"gw{s}", gw_d[s], [128, int(L[s]) // 16], I16)
                  for s in range(2)]
            dw = [cload(f"dw{s}", dw_d[s], [128, int(L[s]) // 16], I16)
                  for s in range(2)]
            drel = [cload(f"drel{s}", drel_d[s], [128, int(tiles[s])], FP16)
                    for s in range(2)]

            # ---------------- phase A: node tables ----------------
            asv_re = asv_d[:].rearrange("(i p) f -> p i f", p=128)
            A_TILES = NPAD // 128
            with tc.tile_pool(name="pa_in", bufs=3) as pa_in, \
                 tc.tile_pool(name="pa_ps", bufs=4, space="PSUM") as pa_ps, \
                 tc.tile_pool(name="pa_out", bufs=3) as pa_out:
                for t0 in range(0, A_TILES, A_CH):
                    ac = min(A_CH, A_TILES - t0)
                    hc = pa_in.tile([128, ac, 128], FP16, tag="hc")
                    nc.sync.dma_start(hc[:], hT_d[:, t0 * 128:(t0 + ac) * 128]
                                      .rearrange("p (i n) -> p i n", i=ac))
                    ao = pa_out.tile([128, ac, 256], FP16, tag="ao")
                    for i in range(ac):
                        mm = pa_ps.tile([128, 256], F32, tag="mm")
                        nc.tensor.matmul(mm[:], hc[:, i, :], Wasv[:],
                                         start=True, stop=True)
                        nc.vector.tensor_copy(ao[:, i, :], mm[:])
                    nc.sync.dma_start(asv_re[:, t0:t0 + ac, :], ao[:])
                adst_re = adst_d[:].rearrange("(i p) f -> p i f", p=128)
                for t0 in range(0, NBLK, A_CH):
                    ac = min(A_CH, NBLK - t0)
                    hc = pa_in.tile([128, ac, 128], FP16, tag="hc")
                    nc.sync.dma_start(hc[:], hloc_d[:, t0 * 128:(t0 + ac) * 128]
                                      .rearrange("p (i n) -> p i n", i=ac))
                    ao = pa_out.tile([128, ac, 128], FP16, tag="ao")
                    for i in range(ac):
                        mm = pa_ps.tile([128, 128], F32, tag="mm")
                        nc.tensor.matmul(mm[:], hc[:, i, :], Wdst[:],
                                         start=True, stop=True)
                        nc.vector.tensor_tensor(ao[:, i, :], mm[:], bias[:],
                                                op=AT.add)
                    nc.sync.dma_start(adst_re[:, t0:t0 + ac, :], ao[:])

            tc.strict_bb_all_engine_barrier()

            # ---------------- phase B + C ----------------
            asv_base = [asv_d[0:split, :], asv_d[split:NPAD, :]]
            chunk_cache = [dict(), dict()]

            with tc.tile_pool(name="pg_asv", bufs=3) as pg_asv, \
                 tc.tile_pool(name="pg_dst", bufs=3) as pg_dst, \
                 tc.tile_pool(name="pb_oh", bufs=3) as pb_oh, \
                 tc.tile_pool(name="pb_s", bufs=2) as pb_s, \
                 tc.tile_pool(name="pb_lr", bufs=2) as pb_lr, \
                 tc.tile_pool(name="pb_pr", bufs=2) as pb_pr, \
                 tc.tile_pool(name="pb_lg", bufs=3) as pb_lg, \
                 tc.tile_pool(name="pb_wt", bufs=3) as pb_wt, \
                 tc.tile_pool(name="ps_agg", bufs=3, space="PSUM") as ps_agg, \
                 tc.tile_pool(name="pc_ps", bufs=2, space="PSUM") as pc_ps, \
                 tc.tile_pool(name="pc_ph", bufs=1, space="PSUM") as pc_ph, \
                 tc.tile_pool(name="pc_sb", bufs=2) as pc_sb:

                def ensure_chunk(s, ci):
                    if ci in chunk_cache[s]:
                        return chunk_cache[s][ci]
                    t0 = ci * G
                    g = min(G, int(tiles[s]) - t0)
                    GSUB = 8  # <=1024 idxs/dma_gather: 64 descs per SDMA
                    # engine, safely under the 128-slot DGE ring
                    asv_g = pg_asv.tile([128, g, 256], FP16, tag="asv_g")
                    dstb_g = pg_dst.tile([128, g, 128], FP16, tag="dstb_g")
                    for k in range(0, g, GSUB):
                        gs = min(GSUB, g - k)
                        ne = gs * 128
                        nc.gpsimd.dma_gather(
                            asv_g[:, k:k + gs, :], asv_base[s],
                            gw[s][:, (t0 + k) * 8:(t0 + k + gs) * 8], ne, ne, 256)
                        nc.gpsimd.dma_gather(
                            dstb_g[:, k:k + gs, :], adst_d[:],
                            dw[s][:, (t0 + k) * 8:(t0 + k + gs) * 8], ne, ne, 128)
                    oh = pb_oh.tile([128, g, 128], FP16, tag="oh")
                    nc.vector.tensor_tensor(
                        oh[:],
                        drel[s][:, t0:t0 + g].rearrange("p (g o) -> p g o", o=1)
                        .to_broadcast((128, g, 128)),
                        iota[:].rearrange("p (o n) -> p o n", o=1)
                        .to_broadcast((128, g, 128)),
                        op=AT.is_equal)
                    st = pb_s.tile([128, g, 128], FP16, tag="st")
                    nc.vector.tensor_tensor(st[:], asv_g[:, :, 0:128], dstb_g[:],
                                            op=AT.add)
                    lr = pb_lr.tile([128, g, 128], FP16, tag="lr")
                    nc.vector.scalar_tensor_tensor(lr[:], st[:], 0.2, st[:],
                                                   op0=AT.mult, op1=AT.max)
                    pr = pb_pr.tile([128, g, 128], FP16, tag="pr")
                    nc.vector.tensor_tensor(
                        pr[:], lr[:],
                        attn[:].rearrange("p (o n) -> p o n", o=1)
                        .to_broadcast((128, g, 128)),
                        op=AT.mult)
                    lg = pb_lg.tile([128, 2 * g], F32, tag="lg")
                    nc.vector.tensor_reduce(
                        lg[:], pr[:].rearrange("p g (h d) -> p (g h) d", h=H),
                        axis=mybir.AxisListType.X, op=AT.add)
                    wt = pb_wt.tile([128, g, 130], FP16, tag="wt")
                    nc.scalar.activation(wt[:, :, 128:130],
                                         lg[:].rearrange("p (g h) -> p g h", h=H),
                                         ACTF.Exp)
                    nc.vector.tensor_tensor(
                        wt[:, :, 0:128].rearrange("p g (h d) -> p g h d", h=H),
                        asv_g[:, :, 128:256].rearrange("p g (h d) -> p g h d", h=H),
                        wt[:, :, 128:130].rearrange("p g (h o) -> p g h o", o=1)
                        .to_broadcast((128, g, H, D)),
                        op=AT.mult)
                    chunk_cache[s][ci] = (oh, wt)
                    return oh, wt

                for j in range(NBLK):
                    n_ev = int(T[0][j] + T[1][j])
                    ps = ps_agg.tile([128, 130], F32, tag="ps")
                    ev = 0
                    for s in range(2):
                        for t in range(int(T[s][j])):
                            gt = int(base_tile[s, j]) + t
                            oh, wt = ensure_chunk(s, gt // G)
                            off = gt % G
                            nc.tensor.matmul(ps[:], oh[:, off, :],
                                             wt[:, off, 0:130],
                                             start=(ev == 0),
                                             stop=(ev == n_ev - 1))
                            ev += 1
                    # ---- phase C for block j ----
                    R = 128 if j < NBLK - 1 else last_rows
                    rc = pc_sb.tile([128, 2], F32, tag="rc")
                    nc.vector.reciprocal(rc[:], ps[:, 128:130])
                    agg = pc_sb.tile([128, 128], F32, tag="agg")
                    if R < 128:
                        nc.vector.memset(agg[:], 0.0)
                    nc.vector.tensor_scalar(agg[0:R, 0:64], ps[0:R, 0:64],
                                            rc[0:R, 0:1], None, op0=AT.mult)
                    nc.vector.tensor_scalar(agg[0:R, 64:128], ps[0:R, 64:128],
                                            rc[0:R, 1:2], None, op0=AT.mult)
                    tp = pc_ps.tile([128, 128], F32, tag="tp")
                    nc.tensor.transpose(tp[:], agg[:], ident_f32[:])
                    aggT = pc_sb.tile([128, 128], FP16, tag="aggT")
                    nc.vector.tensor_copy(aggT[:], tp[:])
                    o1p = pc_ps.tile([128, 128], F32, tag="o1p")
                    nc.tensor.matmul(o1p[:], outW[:], aggT[:], start=True,
                                     stop=True)
                    o1 = pc_sb.tile([128, 128], FP16, tag="o1")
                    nc.vector.tensor_scalar(o1[:], o1p[:], outb[:, 0:1], 0.0,
                                            op0=AT.add, op1=AT.max)
                    php = pc_ph.tile([128, P_OUT], F32, tag="php")
                    nc.tensor.matmul(php[:], o1[:], phW[:], start=True, stop=True)
                    z = pc_sb.tile([128, P_OUT], F32, tag="z")
                    nc.vector.tensor_tensor(z[:], php[:], phb[:], op=AT.add)
                    ez = pc_sb.tile([128, P_OUT], F32, tag="ez")
                    nc.scalar.activation(ez[:], z[:], ACTF.Exp)
                    sm = pc_sb.tile([128, 1], F32, tag="sm")
                    nc.vector.tensor_reduce(sm[:], ez[:],
                                            axis=mybir.AxisListType.X, op=AT.add)
                    rc2 = pc_sb.tile([128, 1], F32, tag="rc2")
                    nc.vector.reciprocal(rc2[:], sm[:])
                    ot = pc_sb.tile([128, P_OUT], F32, tag="ot")
                    nc.vector.tensor_scalar(ot[:], ez[:], rc2[:, 0:1], None,
                                            op0=AT.mult)
                    nc.sync.dma_start(out_d[j * 128:j * 128 + R, :], ot[0:R, :])

    nc.compile()
    return nc


_CACHE = {}


def kernel(**inputs) -> np.ndarray:
    meta, in_maps = prep(**inputs)
    key = "nc"
    if key not in _CACHE:
        _CACHE[key] = build(meta)
    nc = _CACHE[key]
    res = run_bass_kernel_spmd(nc, in_maps, core_ids=list(range(N_CORES)))
    out = np.concatenate([res.results[c]["out"] for c in range(N_CORES)],
                         axis=0)
    return out.astype(np.float32)



# revision 6
# speedup vs baseline: 1.4856x; 1.4856x over previous
"""GATv2 actor layer (nn_GATv2Actor) on 8 TRN2 NeuronCores via Bass/Tile.

Self-contained: kernel(**inputs) takes the full (unsharded) inputs of
reference.setup_inputs() and returns the full [50000, 4] float32 output.

Distribution (edge-parallel by destination-node range): node n is owned by
core n // 6250; each core handles all edges whose destination lies in its
range (plus its self-loops), so the segment softmax and scatter-add are
core-local and output rows are disjoint (host concatenates).

Math restructure vs the straightforward edge pipeline:
  logits[e,h] = sum_d attn[h,d]*leaky(st[e,h,d])
              = sum_d sign(attn)[h,d] * leaky(|attn|[h,d]*st[e,h,d])
  (leaky_relu is positively homogeneous), so |attn| is folded into pair_W/
  pair_b on the host and the weighted reduce becomes a 2-column TensorE
  matmul against a constant +-1 sign mask.

Per-core phases:
  A: node tables. asv[n] = [|attn|-scaled h@pair_W_src | h@value_W] (fp16,
     512B rows, DRAM, gathered by edge src); adst[n] (dst half + bias,
     SBUF-resident per core).
  B: edges sorted by (dst, src), 128/partition-dim tiles, two streams by
     src<32768 (int16 gather indices). Per tile:
       - dma_gather asv rows (Pool desc-gen, 512B rows - no small-desc
         penalty on the shared DMA resource)
       - one-hot(n on partitions) oh_n_e built by DVE TensorTensor against
         a host-streamed partition-broadcast drel (plain packed operands
         keep the 2x DVE mode; AP broadcasts would drop to f32 rate)
       - st^T[d,e] in PSUM via two TensorE matmuls: identity-moving
         transpose of the gathered att half + adst_blk @ oh_n_e
       - leaky-relu on the Scalar engine (PSUM-in, 8-tile groups)
       - logits = matmul(lr_T, signmask) accumulated per-chunk in PSUM
       - exp on Scalar engine (fp16 copy into wt cols 128:130 + f32 copy
         for scaling), wt = v * ex via 4x-mode tensor_scalar per head
       - one-hot(e on partitions) oh_e_n via 4x-mode tensor_scalar
         is_equal(iota, drel) and a scatter matmul accumulating
         [ex-weighted values | ex] into the block's PSUM accumulator
  C: per 128-node block: agg = U/denom (fused divide), output MLP + phase
     softmax, DMA out.

SPMD: one program for all 8 cores; per-(stream,block) tile counts padded to
the max over cores; padded edges carry drel=-1 so their one-hot columns are
zero and they contribute nothing.
"""
import math
import sys

import numpy as np

sys.path.insert(0, "/opt/trn_rl_repo")

import concourse.bass as bass  # noqa: E402
import concourse.tile as tile  # noqa: E402
from concourse import bacc, mybir  # noqa: E402
from concourse.bass_utils import run_bass_kernel_spmd  # noqa: E402

FP16 = mybir.dt.float16
F32 = mybir.dt.float32
I16 = mybir.dt.int16
AT = mybir.AluOpType
ACTF = mybir.ActivationFunctionType

F = 128      # feature dim
H = 2        # heads
D = 64       # head dim
P_OUT = 4    # phases
N_CORES = 8


def prep(h_int, edge_index, pair_W, pair_b, attn_w, value_W, out_W, out_b,
         phase_W, phase_b, n_cores=N_CORES, G=24, split=32768, A_CH=8):
    """Host-side index preprocessing + input packing. Returns (meta, in_maps)."""
    h = np.asarray(h_int, np.float32)
    ei = np.asarray(edge_index)
    pair_W = np.asarray(pair_W, np.float32)
    pair_b = np.asarray(pair_b, np.float32)
    attn_w = np.asarray(attn_w, np.float32)
    value_W = np.asarray(value_W, np.float32)
    out_W = np.asarray(out_W, np.float32)
    out_b = np.asarray(out_b, np.float32)
    phase_W = np.asarray(phase_W, np.float32)
    phase_b = np.asarray(phase_b, np.float32)
    N = h.shape[0]
    assert N % n_cores == 0
    NPC = N // n_cores
    NBLK = (NPC + 127) // 128
    NPAD = ((N + 127) // 128) * 128
    assert NPAD - split < 32768 and split < 32768 + 1

    src = np.concatenate([ei[0], np.arange(N)]).astype(np.int64)
    dst = np.concatenate([ei[1], np.arange(N)]).astype(np.int64)
    core = dst // NPC

    percore = []
    counts = np.zeros((n_cores, 2, NBLK), np.int64)
    for c in range(n_cores):
        m = core == c
        es = src[m]
        ed = dst[m] - c * NPC
        o = np.lexsort((es, ed))
        es, ed = es[o], ed[o]
        lo = es < split
        percore.append((es, ed, lo))
        for si in range(2):
            msk = lo if si == 0 else ~lo
            counts[c, si] = np.bincount(ed[msk] // 128, minlength=NBLK)
    T = np.ceil(counts.max(axis=0) / 128.0).astype(np.int64)  # [2, NBLK]
    tiles = T.sum(axis=1)
    L = tiles * 128
    base_tile = np.zeros((2, NBLK + 1), np.int64)
    base_tile[:, 1:] = np.cumsum(T, axis=1)

    f16 = np.float16
    aw = np.abs(attn_w)                      # [H, D] magnitudes
    sg = np.sign(attn_w).astype(np.float32)  # [H, D] signs (+-1 or 0)
    # |attn|-folded weight halves
    Wsrc0 = pair_W[0, :F] * aw[0][None, :]
    Wsrc1 = pair_W[1, :F] * aw[1][None, :]
    Wdst0 = pair_W[0, F:] * aw[0][None, :]
    Wdst1 = pair_W[1, F:] * aw[1][None, :]
    W_asv = np.concatenate([Wsrc0, Wsrc1, value_W[0], value_W[1]],
                           axis=1).astype(f16)
    W_dst = np.concatenate([Wdst0, Wdst1], axis=1).astype(f16)
    bias_sc = (pair_b * aw).reshape(-1)      # [128] |attn|-scaled bias
    bias_bc = np.broadcast_to(bias_sc.astype(np.float32), (128, F)).copy()
    # sign mask [128, 2]: row d, col h = sign(attn[h, d-64h]) if d in head h
    signmask = np.zeros((128, H), np.float32)
    signmask[0:64, 0] = sg[0]
    signmask[64:128, 1] = sg[1]
    signmask = signmask.astype(f16)
    iota_bc = np.broadcast_to(np.arange(128, dtype=f16), (128, 128)).copy()
    iota_col = np.broadcast_to(np.arange(128, dtype=f16)[:, None],
                               (128, G * 128)).copy()
    ident16 = np.eye(128, dtype=f16)
    out_Wt = np.asarray(out_W, f16)
    out_b_c = np.asarray(out_b, np.float32).reshape(128, 1).copy()
    phase_Wt = np.asarray(phase_W, f16)
    phase_b_bc = np.broadcast_to(np.asarray(phase_b, np.float32),
                                 (128, P_OUT)).copy()

    hp = np.zeros((NPAD, F), np.float32)
    hp[:N] = h
    hT16 = np.ascontiguousarray(hp.T.astype(f16))

    shared = dict(hT16=hT16, W_asv=W_asv, W_dst=W_dst, bias_bc=bias_bc,
                  signmask=signmask, iota_bc=iota_bc, iota_col=iota_col,
                  ident16=ident16, out_Wt=out_Wt, out_b=out_b_c,
                  phase_Wt=phase_Wt, phase_b_bc=phase_b_bc)

    in_maps = []
    for c in range(n_cores):
        es, ed, lo = percore[c]
        m = {}
        for si in range(2):
            msk = lo if si == 0 else ~lo
            es_s, ed_s = es[msk], ed[msk]
            gidx = np.zeros(L[si], np.int16)
            drel = np.full(L[si], -1.0, np.float32)
            starts = np.searchsorted(ed_s, np.arange(NBLK + 1) * 128)
            for j in range(NBLK):
                seg = slice(starts[j], starts[j + 1])
                n = starts[j + 1] - starts[j]
                b0 = base_tile[si, j] * 128
                gidx[b0:b0 + n] = (es_s[seg] - (split if si else 0)).astype(np.int16)
                drel[b0:b0 + n] = (ed_s[seg] - j * 128).astype(np.float32)
            m[f"gw{si}"] = np.tile(gidx.reshape(-1, 16).T, (8, 1)).copy()
            m[f"drf{si}"] = np.ascontiguousarray(drel.reshape(-1, 128).T)
            m[f"drbc{si}"] = np.broadcast_to(
                drel.astype(f16)[None, :], (128, int(L[si]))).copy()
        hl = np.zeros((NBLK * 128, F), np.float32)
        hl[:NPC] = h[c * NPC:(c + 1) * NPC]
        m["hlocT16"] = np.ascontiguousarray(hl.T.astype(f16))
        m.update(shared)
        in_maps.append(m)

    meta = dict(N=N, NPC=NPC, NBLK=NBLK, NPAD=NPAD, split=split, G=G,
                A_CH=A_CH, T=T, tiles=tiles, L=L, base_tile=base_tile,
                n_cores=n_cores)
    return meta, in_maps


def build(meta):
    NPC, NBLK, NPAD = meta["NPC"], meta["NBLK"], meta["NPAD"]
    split, G, A_CH = meta["split"], meta["G"], meta["A_CH"]
    T, tiles, L = meta["T"], meta["tiles"], meta["L"]
    base_tile = meta["base_tile"]
    last_rows = NPC - (NBLK - 1) * 128

    # tile -> block id per stream (static schedule)
    tile_block = [[], []]
    for s in range(2):
        for j in range(NBLK):
            tile_block[s].extend([j] * int(T[s][j]))

    nc = bacc.Bacc(None, target_bir_lowering=False, debug=False)

    hT_d = nc.dram_tensor("hT16", [128, NPAD], FP16, kind="ExternalInput")
    hloc_d = nc.dram_tensor("hlocT16", [128, NBLK * 128], FP16,
                            kind="ExternalInput")
    gw_d = [nc.dram_tensor(f"gw{s}", [128, int(L[s]) // 16], I16,
                           kind="ExternalInput") for s in range(2)]
    drf_d = [nc.dram_tensor(f"drf{s}", [128, int(tiles[s])], F32,
                            kind="ExternalInput") for s in range(2)]
    drbc_d = [nc.dram_tensor(f"drbc{s}", [128, int(L[s])], FP16,
                             kind="ExternalInput") for s in range(2)]
    Wasv_d = nc.dram_tensor("W_asv", [128, 256], FP16, kind="ExternalInput")
    Wdst_d = nc.dram_tensor("W_dst", [128, 128], FP16, kind="ExternalInput")
    bias_d = nc.dram_tensor("bias_bc", [128, 128], F32, kind="ExternalInput")
    sgn_d = nc.dram_tensor("signmask", [128, H], FP16, kind="ExternalInput")
    iota_d = nc.dram_tensor("iota_bc", [128, 128], FP16, kind="ExternalInput")
    iotac_d = nc.dram_tensor("iota_col", [128, G * 128], FP16,
                             kind="ExternalInput")
    ident_d = nc.dram_tensor("ident16", [128, 128], FP16, kind="ExternalInput")
    outW_d = nc.dram_tensor("out_Wt", [128, 128], FP16, kind="ExternalInput")
    outb_d = nc.dram_tensor("out_b", [128, 1], F32, kind="ExternalInput")
    phW_d = nc.dram_tensor("phase_Wt", [128, P_OUT], FP16, kind="ExternalInput")
    phb_d = nc.dram_tensor("phase_b_bc", [128, P_OUT], F32, kind="ExternalInput")

    asv_d = nc.dram_tensor("asv_tab", [NPAD, 256], FP16)
    out_d = nc.dram_tensor("out", [NPC, P_OUT], F32, kind="ExternalOutput")

    with tile.TileContext(nc) as tc:
        with tc.tile_pool(name="consts", bufs=1) as pc:
            def cload(name, dram, shape, dtype):
                t = pc.tile(shape, dtype, tag=name, name=name)
                nc.sync.dma_start(t[:], dram[:])
                return t
            Wasv = cload("Wasv", Wasv_d, [128, 256], FP16)
            Wdst = cload("Wdst", Wdst_d, [128, 128], FP16)
            bias = cload("bias", bias_d, [128, 128], F32)
            sgn = cload("sgn", sgn_d, [128, H], FP16)
            iota = cload("iota", iota_d, [128, 128], FP16)
            iotac = cload("iotac", iotac_d, [128, G * 128], FP16)
            ident = cload("ident", ident_d, [128, 128], FP16)
            outW = cload("outW", outW_d, [128, 128], FP16)
            outb = cload("outb", outb_d, [128, 1], F32)
            phW = cload("phW", phW_d, [128, P_OUT], FP16)
            phb = cload("phb", phb_d, [128, P_OUT], F32)
            gw = [cload(f"gw{s}", gw_d[s], [128, int(L[s]) // 16], I16)
                  for s in range(2)]
            drf = [cload(f"drf{s}", drf_d[s], [128, int(tiles[s])], F32)
                   for s in range(2)]
            # adst table: SBUF-resident per core [128, NBLK, 128] fp16
            adst_res = pc.tile([128, NBLK, 128], FP16, tag="adst_res",
                               name="adst_res")

            # ---------------- phase A: node tables ----------------
            asv_re = asv_d[:].rearrange("(i p) f -> p i f", p=128)
            A_TILES = NPAD // 128
            with tc.tile_pool(name="pa_in", bufs=3) as pa_in, \
                 tc.tile_pool(name="pa_ps", bufs=2, space="PSUM") as pa_ps, \
                 tc.tile_pool(name="pa_out", bufs=3) as pa_out:
                copy_flip = 0
                for t0 in range(0, A_TILES, A_CH):
                    ac = min(A_CH, A_TILES - t0)
                    hc = pa_in.tile([128, A_CH, 128], FP16, tag="hc", name="hc")
                    nc.sync.dma_start(hc[:, 0:ac, :],
                                      hT_d[:, t0 * 128:(t0 + ac) * 128]
                                      .rearrange("p (i n) -> p i n", i=ac))
                    mm = pa_ps.tile([128, A_CH, 256], F32, tag="mm", name="mm")
                    for i in range(ac):
                        nc.tensor.matmul(mm[:, i, :], hc[:, i, :], Wasv[:],
                                         start=True, stop=True)
                    ao = pa_out.tile([128, A_CH, 256], FP16, tag="ao", name="ao")
                    eng = nc.scalar if (copy_flip % 4) else nc.vector
                    copy_flip += 1
                    if eng is nc.scalar:
                        nc.scalar.copy(ao[:, 0:ac, :], mm[:, 0:ac, :])
                    else:
                        nc.vector.tensor_copy(ao[:, 0:ac, :], mm[:, 0:ac, :])
                    nc.sync.dma_start(asv_re[:, t0:t0 + ac, :], ao[:, 0:ac, :])
                for t0 in range(0, NBLK, A_CH):
                    ac = min(A_CH, NBLK - t0)
                    hc = pa_in.tile([128, A_CH, 128], FP16, tag="hc", name="hc2")
                    nc.sync.dma_start(hc[:, 0:ac, :],
                                      hloc_d[:, t0 * 128:(t0 + ac) * 128]
                                      .rearrange("p (i n) -> p i n", i=ac))
                    mm = pa_ps.tile([128, A_CH, 256], F32, tag="mm", name="mm2")
                    for i in range(ac):
                        nc.tensor.matmul(mm[:, i, 0:128], hc[:, i, :], Wdst[:],
                                         start=True, stop=True)
                    with nc.allow_low_precision(reason="fp16 edge tables"):
                        nc.vector.tensor_tensor(
                            adst_res[:, t0:t0 + ac, :], mm[:, 0:ac, 0:128],
                            bias[:].rearrange("p (o n) -> p o n", o=1)
                            .to_broadcast((128, ac, 128)),
                            op=AT.add)

            tc.strict_bb_all_engine_barrier()

            # ---------------- phase B + C ----------------
            asv_base = [asv_d[0:split, :], asv_d[split:NPAD, :]]
            chunk_cache = [dict(), dict()]

            with tc.tile_pool(name="pg_asv", bufs=2) as pg_asv, \
                 tc.tile_pool(name="pg_dr", bufs=2) as pg_dr, \
                 tc.tile_pool(name="pb_ohne", bufs=2) as pb_ohne, \
                 tc.tile_pool(name="pb_ohen", bufs=2) as pb_ohen, \
                 tc.tile_pool(name="pb_lr", bufs=2) as pb_lr, \
                 tc.tile_pool(name="pb_wt", bufs=2) as pb_wt, \
                 tc.tile_pool(name="pb_exf", bufs=2) as pb_exf, \
                 tc.tile_pool(name="ps_st", bufs=2, space="PSUM") as ps_st, \
                 tc.tile_pool(name="ps_lg", bufs=1, space="PSUM") as ps_lg, \
                 tc.tile_pool(name="ps_agg", bufs=2, space="PSUM") as ps_agg, \
                 tc.tile_pool(name="pc_ps", bufs=2, space="PSUM") as pc_ps, \
                 tc.tile_pool(name="pc_sb", bufs=2) as pc_sb:

                def ensure_chunk(s, ci):
                    if ci in chunk_cache[s]:
                        return chunk_cache[s][ci]
                    t0 = ci * G
                    g = min(G, int(tiles[s]) - t0)
                    GSUB = 8  # <=1024 idxs/dma_gather: 64 descs per SDMA
                    # engine, safely under the 128-slot DGE ring
                    asv_g = pg_asv.tile([128, G, 256], FP16, tag="asv_g",
                                        name="asv_g")
                    for k in range(0, g, GSUB):
                        gs = min(GSUB, g - k)
                        ne = gs * 128
                        nc.gpsimd.dma_gather(
                            asv_g[:, k:k + gs, :], asv_base[s],
                            gw[s][:, (t0 + k) * 8:(t0 + k + gs) * 8],
                            ne, ne, 256)
                    drbc = pg_dr.tile([128, G, 128], FP16, tag="drbc",
                                      name="drbc")
                    nc.sync.dma_start(
                        drbc[:, 0:g, :],
                        drbc_d[s][:, t0 * 128:(t0 + g) * 128]
                        .rearrange("p (t e) -> p t e", t=g))
                    # one-hot with n on partitions (for a_dst matmul)
                    ohne = pb_ohne.tile([128, G, 128], FP16, tag="ohne",
                                        name="ohne")
                    nc.vector.tensor_tensor(
                        ohne[:, 0:g, :], drbc[:, 0:g, :],
                        iotac[:].rearrange("p (t e) -> p t e", t=G)[:, 0:g, :],
                        op=AT.is_equal)
                    # st^T per tile into PSUM (8-tile bank groups)
                    lr = pb_lr.tile([128, G, 128], FP16, tag="lr", name="lr")
                    lgp = ps_lg.tile([128, G, H], F32, tag="lgp", name="lgp")
                    for k in range(0, g, 4):
                        gs = min(4, g - k)
                        stp = ps_st.tile([128, 4, 128], F32, tag="stp",
                                         name="stp")
                        for i in range(gs):
                            t = k + i
                            j = tile_block[s][t0 + t]
                            nc.tensor.matmul(stp[:, i, :],
                                             asv_g[:, t, 0:128], ident[:],
                                             start=True, stop=False)
                            nc.tensor.matmul(stp[:, i, :],
                                             adst_res[:, j, :],
                                             ohne[:, t, :],
                                             start=False, stop=True)
                        nc.scalar.activation(lr[:, k:k + gs, :],
                                             stp[:, 0:gs, :], ACTF.Lrelu,
                                             alpha=0.2)
                        for i in range(gs):
                            t = k + i
                            nc.tensor.matmul(lgp[:, t, :], lr[:, t, :],
                                             sgn[:], start=True, stop=True)
                    # exp -> fp16 into wt cols 128:130 and f32 scratch
                    wt = pb_wt.tile([128, G, 130], FP16, tag="wt", name="wt")
                    exf = pb_exf.tile([128, G, H], F32, tag="exf", name="exf")
                    nc.scalar.activation(wt[:, 0:g, 128:130], lgp[:, 0:g, :],
                                         ACTF.Exp)
                    nc.scalar.activation(exf[:, 0:g, :], lgp[:, 0:g, :],
                                         ACTF.Exp)
                    # wt = v * ex (per tile per head, 4x tensor_scalar) and
                    # one-hot with e on partitions (for scatter matmul)
                    ohen = pb_ohen.tile([128, G, 128], FP16, tag="ohen",
                                        name="ohen")
                    with nc.allow_low_precision(reason="fp16 edge math"):
                        for t in range(g):
                            for hh in range(H):
                                nc.vector.tensor_scalar(
                                    wt[:, t, hh * 64:(hh + 1) * 64],
                                    asv_g[:, t, 128 + hh * 64:128 + (hh + 1) * 64],
                                    exf[:, t, hh:hh + 1], None, op0=AT.mult)
                            nc.vector.tensor_scalar(
                                ohen[:, t, :], iota[:],
                                drf[s][:, t0 + t:t0 + t + 1], None,
                                op0=AT.is_equal)
                    chunk_cache[s][ci] = (ohen, wt)
                    return ohen, wt

                for j in range(NBLK):
                    n_ev = int(T[0][j] + T[1][j])
                    ps = ps_agg.tile([128, 130], F32, tag="ps", name="ps")
                    ev = 0
                    for s in range(2):
                        for t in range(int(T[s][j])):
                            gt = int(base_tile[s, j]) + t
                            ohen, wt = ensure_chunk(s, gt // G)
                            off = gt % G
                            nc.tensor.matmul(ps[:], ohen[:, off, :],
                                             wt[:, off, 0:130],
                                             start=(ev == 0),
                                             stop=(ev == n_ev - 1))
                            ev += 1
                    # ---- phase C for block j ----
                    R = 128 if j < NBLK - 1 else last_rows
                    rc = pc_sb.tile([128, H], F32, tag="rc", name="rc")
                    nc.vector.reciprocal(rc[:], ps[:, 128:130])
                    agg = pc_sb.tile([128, 128], FP16, tag="agg", name="agg")
                    with nc.allow_low_precision(reason="fp16 mlp"):
                        if R < 128:
                            nc.vector.memset(agg[:], 0.0)
                        nc.vector.tensor_tensor(
                            agg[0:R, :].rearrange("p (h d) -> p h d", h=H),
                            ps[0:R, 0:128].rearrange("p (h d) -> p h d", h=H),
                            rc[0:R, :].rearrange("p (h o) -> p h o", o=1)
                            .to_broadcast((R, H, D)),
                            op=AT.mult)
                    tp = pc_ps.tile([128, 128], F32, tag="cps", name="tp")
                    nc.tensor.matmul(tp[:], agg[:], ident[:], start=True,
                                     stop=True)
                    aggT = pc_sb.tile([128, 128], FP16, tag="aggT", name="aggT")
                    nc.scalar.copy(aggT[:], tp[:])
                    o1p = pc_ps.tile([128, 128], F32, tag="cps", name="o1p")
                    nc.tensor.matmul(o1p[:], outW[:], aggT[:], start=True,
                                     stop=True)
                    o1 = pc_sb.tile([128, 128], FP16, tag="o1", name="o1")
                    nc.scalar.activation(o1[:], o1p[:], ACTF.Relu,
                                         bias=outb[:, 0:1])
                    phpt = pc_ps.tile([128, 128], F32, tag="cps", name="phpt")
                    php = phpt[:, 0:P_OUT]
                    nc.tensor.matmul(php, o1[:], phW[:], start=True,
                                     stop=True)
                    z = pc_sb.tile([128, P_OUT], F32, tag="z", name="z")
                    nc.vector.tensor_tensor(z[:], php, phb[:], op=AT.add)
                    ez = pc_sb.tile([128, P_OUT], F32, tag="ez", name="ez")
                    nc.scalar.activation(ez[:], z[:], ACTF.Exp)
                    sm = pc_sb.tile([128, 1], F32, tag="sm", name="sm")
                    nc.vector.tensor_reduce(sm[:], ez[:],
                                            axis=mybir.AxisListType.X, op=AT.add)
                    rc2 = pc_sb.tile([128, 1], F32, tag="rc2", name="rc2")
                    nc.vector.reciprocal(rc2[:], sm[:])
                    ot = pc_sb.tile([128, P_OUT], F32, tag="ot", name="ot")
                    nc.vector.tensor_scalar(ot[:], ez[:], rc2[:, 0:1], None,
                                            op0=AT.mult)
                    nc.sync.dma_start(out_d[j * 128:j * 128 + R, :], ot[0:R, :])

    nc.compile()
    return nc


_CACHE = {}


def kernel(**inputs) -> np.ndarray:
    meta, in_maps = prep(**inputs)
    key = "nc"
    if key not in _CACHE:
        _CACHE[key] = build(meta)
    nc = _CACHE[key]
    res = run_bass_kernel_spmd(nc, in_maps, core_ids=list(range(N_CORES)))
    out = np.concatenate([res.results[c]["out"] for c in range(N_CORES)],
                         axis=0)
    return out.astype(np.float32)


# revision 7
# speedup vs baseline: 1.8471x; 1.2434x over previous
"""GATv2 actor layer (nn_GATv2Actor) on 8 TRN2 NeuronCores via Bass/Tile.

Self-contained: kernel(**inputs) takes the full (unsharded) inputs of
reference.setup_inputs() and returns the full [50000, 4] float32 output.

Distribution (edge-parallel by destination-node range): node n is owned by
core n // 6250; each core handles all edges whose destination lies in its
range (plus its self-loops), so the segment softmax and scatter-add are
core-local and output rows are disjoint (host concatenates).

Math restructure vs the straightforward edge pipeline:
  logits[e,h] = sum_d attn[h,d]*leaky(st[e,h,d])
              = sum_d sign(attn)[h,d] * leaky(|attn|[h,d]*st[e,h,d])
  (leaky_relu is positively homogeneous), so |attn| is folded into pair_W/
  pair_b on the host and the weighted reduce becomes a 2-column TensorE
  matmul against a constant +-1 sign mask.

Per-core phases:
  A: node tables. asv[n] = [|attn|-scaled h@pair_W_src | h@value_W] (fp16,
     512B rows, DRAM, gathered by edge src); adst[n] (dst half + bias,
     SBUF-resident per core).
  B: edges sorted by (dst, src), 128/partition-dim tiles, two streams by
     src<32768 (int16 gather indices). Per tile:
       - dma_gather asv rows (Pool desc-gen, 512B rows - no small-desc
         penalty on the shared DMA resource)
       - one-hot(n on partitions) oh_n_e built by DVE TensorTensor against
         a host-streamed partition-broadcast drel (plain packed operands
         keep the 2x DVE mode; AP broadcasts would drop to f32 rate)
       - st^T[d,e] in PSUM via two TensorE matmuls: identity-moving
         transpose of the gathered att half + adst_blk @ oh_n_e
       - leaky-relu on the Scalar engine (PSUM-in, 8-tile groups)
       - logits = matmul(lr_T, signmask) accumulated per-chunk in PSUM
       - exp on Scalar engine (fp16 copy into wt cols 128:130 + f32 copy
         for scaling), wt = v * ex via 4x-mode tensor_scalar per head
       - one-hot(e on partitions) oh_e_n via 4x-mode tensor_scalar
         is_equal(iota, drel) and a scatter matmul accumulating
         [ex-weighted values | ex] into the block's PSUM accumulator
  C: per 128-node block: agg = U/denom (fused divide), output MLP + phase
     softmax, DMA out.

SPMD: one program for all 8 cores; per-(stream,block) tile counts padded to
the max over cores; padded edges carry drel=-1 so their one-hot columns are
zero and they contribute nothing.
"""
import math
import sys

import numpy as np

sys.path.insert(0, "/opt/trn_rl_repo")

import concourse.bass as bass  # noqa: E402
import concourse.tile as tile  # noqa: E402
from concourse import bacc, mybir  # noqa: E402
from concourse.bass_utils import run_bass_kernel_spmd  # noqa: E402

FP16 = mybir.dt.float16
F32 = mybir.dt.float32
I16 = mybir.dt.int16
AT = mybir.AluOpType
ACTF = mybir.ActivationFunctionType

F = 128      # feature dim
H = 2        # heads
D = 64       # head dim
P_OUT = 4    # phases
N_CORES = 8


def prep(h_int, edge_index, pair_W, pair_b, attn_w, value_W, out_W, out_b,
         phase_W, phase_b, n_cores=N_CORES, G=24, split=32768, A_CH=8):
    """Host-side index preprocessing + input packing. Returns (meta, in_maps)."""
    h = np.asarray(h_int, np.float32)
    ei = np.asarray(edge_index)
    pair_W = np.asarray(pair_W, np.float32)
    pair_b = np.asarray(pair_b, np.float32)
    attn_w = np.asarray(attn_w, np.float32)
    value_W = np.asarray(value_W, np.float32)
    out_W = np.asarray(out_W, np.float32)
    out_b = np.asarray(out_b, np.float32)
    phase_W = np.asarray(phase_W, np.float32)
    phase_b = np.asarray(phase_b, np.float32)
    N = h.shape[0]
    assert N % n_cores == 0
    NPC = N // n_cores
    NBLK = (NPC + 127) // 128
    NPAD = ((N + 127) // 128) * 128
    assert NPAD - split < 32768 and split < 32768 + 1

    src = np.concatenate([ei[0], np.arange(N)]).astype(np.int64)
    dst = np.concatenate([ei[1], np.arange(N)]).astype(np.int64)
    core = dst // NPC

    percore = []
    counts = np.zeros((n_cores, 2, NBLK), np.int64)
    for c in range(n_cores):
        m = core == c
        es = src[m]
        ed = dst[m] - c * NPC
        o = np.lexsort((es, ed))
        es, ed = es[o], ed[o]
        lo = es < split
        percore.append((es, ed, lo))
        for si in range(2):
            msk = lo if si == 0 else ~lo
            counts[c, si] = np.bincount(ed[msk] // 128, minlength=NBLK)
    T = np.ceil(counts.max(axis=0) / 128.0).astype(np.int64)  # [2, NBLK]
    tiles = T.sum(axis=1)
    L = tiles * 128
    base_tile = np.zeros((2, NBLK + 1), np.int64)
    base_tile[:, 1:] = np.cumsum(T, axis=1)

    f16 = np.float16
    aw = np.abs(attn_w)                      # [H, D] magnitudes
    sg = np.sign(attn_w).astype(np.float32)  # [H, D] signs (+-1 or 0)
    # |attn|-folded weight halves
    Wsrc0 = pair_W[0, :F] * aw[0][None, :]
    Wsrc1 = pair_W[1, :F] * aw[1][None, :]
    Wdst0 = pair_W[0, F:] * aw[0][None, :]
    Wdst1 = pair_W[1, F:] * aw[1][None, :]
    W_asv = np.concatenate([Wsrc0, Wsrc1, value_W[0], value_W[1]],
                           axis=1).astype(f16)
    W_dst = np.concatenate([Wdst0, Wdst1], axis=1).astype(f16)
    bias_sc = (pair_b * aw).reshape(-1)      # [128] |attn|-scaled bias
    bias_bc = np.broadcast_to(bias_sc.astype(np.float32), (128, F)).copy()
    # sign mask [128, 2]: row d, col h = sign(attn[h, d-64h]) if d in head h
    signmask = np.zeros((128, H), np.float32)
    signmask[0:64, 0] = sg[0]
    signmask[64:128, 1] = sg[1]
    signmask = signmask.astype(f16)
    iota_bc = np.broadcast_to(np.arange(128, dtype=f16), (128, 128)).copy()
    iota_col = np.broadcast_to(np.arange(128, dtype=f16)[:, None],
                               (128, G * 128)).copy()
    ident16 = np.eye(128, dtype=f16)
    out_Wt = np.asarray(out_W, f16)
    out_b_c = np.asarray(out_b, np.float32).reshape(128, 1).copy()
    phase_Wt = np.asarray(phase_W, f16)
    phase_b_bc = np.broadcast_to(np.asarray(phase_b, np.float32),
                                 (128, P_OUT)).copy()

    hp = np.zeros((NPAD, F), np.float32)
    hp[:N] = h
    hT16 = np.ascontiguousarray(hp.T.astype(f16))

    shared = dict(hT16=hT16, W_asv=W_asv, W_dst=W_dst, bias_bc=bias_bc,
                  signmask=signmask, iota_bc=iota_bc, iota_col=iota_col,
                  ident16=ident16, out_Wt=out_Wt, out_b=out_b_c,
                  phase_Wt=phase_Wt, phase_b_bc=phase_b_bc)

    in_maps = []
    for c in range(n_cores):
        es, ed, lo = percore[c]
        m = {}
        for si in range(2):
            msk = lo if si == 0 else ~lo
            es_s, ed_s = es[msk], ed[msk]
            gidx = np.zeros(L[si], np.int16)
            drel = np.full(L[si], -1.0, np.float32)
            starts = np.searchsorted(ed_s, np.arange(NBLK + 1) * 128)
            for j in range(NBLK):
                seg = slice(starts[j], starts[j + 1])
                n = starts[j + 1] - starts[j]
                b0 = base_tile[si, j] * 128
                gidx[b0:b0 + n] = (es_s[seg] - (split if si else 0)).astype(np.int16)
                drel[b0:b0 + n] = (ed_s[seg] - j * 128).astype(np.float32)
            m[f"gw{si}"] = np.tile(gidx.reshape(-1, 16).T, (8, 1)).copy()
            m[f"drf{si}"] = np.ascontiguousarray(drel.reshape(-1, 128).T)
            m[f"drbc{si}"] = np.broadcast_to(
                drel.astype(f16)[None, :], (128, int(L[si]))).copy()
        hl = np.zeros((NBLK * 128, F), np.float32)
        hl[:NPC] = h[c * NPC:(c + 1) * NPC]
        m["hlocT16"] = np.ascontiguousarray(hl.T.astype(f16))
        m.update(shared)
        in_maps.append(m)

    meta = dict(N=N, NPC=NPC, NBLK=NBLK, NPAD=NPAD, split=split, G=G,
                A_CH=A_CH, T=T, tiles=tiles, L=L, base_tile=base_tile,
                n_cores=n_cores)
    return meta, in_maps


def build(meta):
    NPC, NBLK, NPAD = meta["NPC"], meta["NBLK"], meta["NPAD"]
    split, G, A_CH = meta["split"], meta["G"], meta["A_CH"]
    T, tiles, L = meta["T"], meta["tiles"], meta["L"]
    base_tile = meta["base_tile"]
    last_rows = NPC - (NBLK - 1) * 128

    # tile -> block id per stream (static schedule)
    tile_block = [[], []]
    for s in range(2):
        for j in range(NBLK):
            tile_block[s].extend([j] * int(T[s][j]))

    nc = bacc.Bacc(None, target_bir_lowering=False, debug=False)

    hT_d = nc.dram_tensor("hT16", [128, NPAD], FP16, kind="ExternalInput")
    hloc_d = nc.dram_tensor("hlocT16", [128, NBLK * 128], FP16,
                            kind="ExternalInput")
    gw_d = [nc.dram_tensor(f"gw{s}", [128, int(L[s]) // 16], I16,
                           kind="ExternalInput") for s in range(2)]
    drf_d = [nc.dram_tensor(f"drf{s}", [128, int(tiles[s])], F32,
                            kind="ExternalInput") for s in range(2)]
    drbc_d = [nc.dram_tensor(f"drbc{s}", [128, int(L[s])], FP16,
                             kind="ExternalInput") for s in range(2)]
    Wasv_d = nc.dram_tensor("W_asv", [128, 256], FP16, kind="ExternalInput")
    Wdst_d = nc.dram_tensor("W_dst", [128, 128], FP16, kind="ExternalInput")
    bias_d = nc.dram_tensor("bias_bc", [128, 128], F32, kind="ExternalInput")
    sgn_d = nc.dram_tensor("signmask", [128, H], FP16, kind="ExternalInput")
    iota_d = nc.dram_tensor("iota_bc", [128, 128], FP16, kind="ExternalInput")
    iotac_d = nc.dram_tensor("iota_col", [128, G * 128], FP16,
                             kind="ExternalInput")
    ident_d = nc.dram_tensor("ident16", [128, 128], FP16, kind="ExternalInput")
    outW_d = nc.dram_tensor("out_Wt", [128, 128], FP16, kind="ExternalInput")
    outb_d = nc.dram_tensor("out_b", [128, 1], F32, kind="ExternalInput")
    phW_d = nc.dram_tensor("phase_Wt", [128, P_OUT], FP16, kind="ExternalInput")
    phb_d = nc.dram_tensor("phase_b_bc", [128, P_OUT], F32, kind="ExternalInput")

    asv_d = nc.dram_tensor("asv_tab", [NPAD, 256], FP16)
    out_d = nc.dram_tensor("out", [NPC, P_OUT], F32, kind="ExternalOutput")

    with tile.TileContext(nc) as tc:
        with tc.tile_pool(name="consts", bufs=1) as pc:
            def cload(name, dram, shape, dtype):
                t = pc.tile(shape, dtype, tag=name, name=name)
                nc.sync.dma_start(t[:], dram[:])
                return t
            Wasv = cload("Wasv", Wasv_d, [128, 256], FP16)
            Wdst = cload("Wdst", Wdst_d, [128, 128], FP16)
            bias = cload("bias", bias_d, [128, 128], F32)
            sgn = cload("sgn", sgn_d, [128, H], FP16)
            iota = cload("iota", iota_d, [128, 128], FP16)
            iotac = cload("iotac", iotac_d, [128, G * 128], FP16)
            ident = cload("ident", ident_d, [128, 128], FP16)
            outW = cload("outW", outW_d, [128, 128], FP16)
            outb = cload("outb", outb_d, [128, 1], F32)
            phW = cload("phW", phW_d, [128, P_OUT], FP16)
            phb = cload("phb", phb_d, [128, P_OUT], F32)
            gw = [cload(f"gw{s}", gw_d[s], [128, int(L[s]) // 16], I16)
                  for s in range(2)]
            drf = [cload(f"drf{s}", drf_d[s], [128, int(tiles[s])], F32)
                   for s in range(2)]
            # adst table: SBUF-resident per core [128, NBLK, 128] fp16
            adst_res = pc.tile([128, NBLK, 128], FP16, tag="adst_res",
                               name="adst_res")

            # ---------------- phase A: node tables ----------------
            asv_re = asv_d[:].rearrange("(i p) f -> p i f", p=128)
            A_TILES = NPAD // 128
            with tc.tile_pool(name="pa_in", bufs=3) as pa_in, \
                 tc.tile_pool(name="pa_ps", bufs=2, space="PSUM") as pa_ps, \
                 tc.tile_pool(name="pa_out", bufs=3) as pa_out:
                copy_flip = 0
                for t0 in range(0, A_TILES, A_CH):
                    ac = min(A_CH, A_TILES - t0)
                    hc = pa_in.tile([128, A_CH, 128], FP16, tag="hc", name="hc")
                    nc.sync.dma_start(hc[:, 0:ac, :],
                                      hT_d[:, t0 * 128:(t0 + ac) * 128]
                                      .rearrange("p (i n) -> p i n", i=ac))
                    mm = pa_ps.tile([128, A_CH, 256], F32, tag="mm", name="mm")
                    for i in range(ac):
                        nc.tensor.matmul(mm[:, i, :], hc[:, i, :], Wasv[:],
                                         start=True, stop=True)
                    ao = pa_out.tile([128, A_CH, 256], FP16, tag="ao", name="ao")
                    eng = nc.scalar if (copy_flip % 4) else nc.vector
                    copy_flip += 1
                    if eng is nc.scalar:
                        nc.scalar.copy(ao[:, 0:ac, :], mm[:, 0:ac, :])
                    else:
                        nc.vector.tensor_copy(ao[:, 0:ac, :], mm[:, 0:ac, :])
                    nc.sync.dma_start(asv_re[:, t0:t0 + ac, :], ao[:, 0:ac, :])
                for t0 in range(0, NBLK, A_CH):
                    ac = min(A_CH, NBLK - t0)
                    hc = pa_in.tile([128, A_CH, 128], FP16, tag="hc", name="hc2")
                    nc.sync.dma_start(hc[:, 0:ac, :],
                                      hloc_d[:, t0 * 128:(t0 + ac) * 128]
                                      .rearrange("p (i n) -> p i n", i=ac))
                    mm = pa_ps.tile([128, A_CH, 256], F32, tag="mm", name="mm2")
                    for i in range(ac):
                        nc.tensor.matmul(mm[:, i, 0:128], hc[:, i, :], Wdst[:],
                                         start=True, stop=True)
                    with nc.allow_low_precision(reason="fp16 edge tables"):
                        nc.vector.tensor_tensor(
                            adst_res[:, t0:t0 + ac, :], mm[:, 0:ac, 0:128],
                            bias[:].rearrange("p (o n) -> p o n", o=1)
                            .to_broadcast((128, ac, 128)),
                            op=AT.add)

            tc.strict_bb_all_engine_barrier()

            # ---------------- phase B + C ----------------
            asv_base = [asv_d[0:split, :], asv_d[split:NPAD, :]]
            chunk_cache = [dict(), dict()]

            with tc.tile_pool(name="pg_asv", bufs=2) as pg_asv, \
                 tc.tile_pool(name="pg_dr", bufs=2) as pg_dr, \
                 tc.tile_pool(name="pb_ohne", bufs=2) as pb_ohne, \
                 tc.tile_pool(name="pb_ohen", bufs=2) as pb_ohen, \
                 tc.tile_pool(name="pb_lr", bufs=2) as pb_lr, \
                 tc.tile_pool(name="pb_wt", bufs=2) as pb_wt, \
                 tc.tile_pool(name="pb_exf", bufs=2) as pb_exf, \
                 tc.tile_pool(name="ps_st", bufs=2, space="PSUM") as ps_st, \
                 tc.tile_pool(name="ps_lg", bufs=1, space="PSUM") as ps_lg, \
                 tc.tile_pool(name="ps_agg", bufs=2, space="PSUM") as ps_agg, \
                 tc.tile_pool(name="pc_ps", bufs=2, space="PSUM") as pc_ps, \
                 tc.tile_pool(name="pc_sb", bufs=2) as pc_sb:

                def ensure_chunk(s, ci):
                    if ci in chunk_cache[s]:
                        return chunk_cache[s][ci]
                    t0 = ci * G
                    g = min(G, int(tiles[s]) - t0)
                    GSUB = 8  # <=1024 idxs/dma_gather: 64 descs per SDMA
                    # engine, safely under the 128-slot DGE ring
                    asv_g = pg_asv.tile([128, G, 256], FP16, tag="asv_g",
                                        name="asv_g")
                    for k in range(0, g, GSUB):
                        gs = min(GSUB, g - k)
                        ne = gs * 128
                        nc.gpsimd.dma_gather(
                            asv_g[:, k:k + gs, :], asv_base[s],
                            gw[s][:, (t0 + k) * 8:(t0 + k + gs) * 8],
                            ne, ne, 256)
                    drbc = pg_dr.tile([128, G, 128], FP16, tag="drbc",
                                      name="drbc")
                    nc.sync.dma_start(
                        drbc[:, 0:g, :],
                        drbc_d[s][:, t0 * 128:(t0 + g) * 128]
                        .rearrange("p (t e) -> p t e", t=g))
                    # one-hot with n on partitions (for a_dst matmul)
                    ohne = pb_ohne.tile([128, G, 128], FP16, tag="ohne",
                                        name="ohne")
                    nc.vector.tensor_tensor(
                        ohne[:, 0:g, :], drbc[:, 0:g, :],
                        iotac[:].rearrange("p (t e) -> p t e", t=G)[:, 0:g, :],
                        op=AT.is_equal)
                    # st^T per tile into PSUM (8-tile bank groups)
                    lr = pb_lr.tile([128, G, 128], FP16, tag="lr", name="lr")
                    lgp = ps_lg.tile([128, G, H], F32, tag="lgp", name="lgp")
                    for k in range(0, g, 4):
                        gs = min(4, g - k)
                        stp = ps_st.tile([128, 4, 128], F32, tag="stp",
                                         name="stp")
                        for i in range(gs):
                            t = k + i
                            j = tile_block[s][t0 + t]
                            nc.tensor.matmul(stp[:, i, :],
                                             asv_g[:, t, 0:128], ident[:],
                                             start=True, stop=False)
                            nc.tensor.matmul(stp[:, i, :],
                                             adst_res[:, j, :],
                                             ohne[:, t, :],
                                             start=False, stop=True)
                        nc.scalar.activation(lr[:, k:k + gs, :],
                                             stp[:, 0:gs, :], ACTF.Prelu,
                                             alpha=0.2)
                        for i in range(gs):
                            t = k + i
                            nc.tensor.matmul(lgp[:, t, :], lr[:, t, :],
                                             sgn[:], start=True, stop=True)
                    # exp -> fp16 into wt cols 128:130 and f32 scratch
                    wt = pb_wt.tile([128, G, 130], FP16, tag="wt", name="wt")
                    exf = pb_exf.tile([128, G, H], F32, tag="exf", name="exf")
                    nc.scalar.activation(wt[:, 0:g, 128:130], lgp[:, 0:g, :],
                                         ACTF.Exp)
                    nc.scalar.activation(exf[:, 0:g, :], lgp[:, 0:g, :],
                                         ACTF.Exp)
                    # wt = v * ex (per tile per head, 4x tensor_scalar) and
                    # one-hot with e on partitions (for scatter matmul)
                    ohen = pb_ohen.tile([128, G, 128], FP16, tag="ohen",
                                        name="ohen")
                    with nc.allow_low_precision(reason="fp16 edge math"):
                        for t in range(g):
                            for hh in range(H):
                                nc.vector.tensor_scalar(
                                    wt[:, t, hh * 64:(hh + 1) * 64],
                                    asv_g[:, t, 128 + hh * 64:128 + (hh + 1) * 64],
                                    exf[:, t, hh:hh + 1], None, op0=AT.mult)
                            nc.vector.tensor_scalar(
                                ohen[:, t, :], iota[:],
                                drf[s][:, t0 + t:t0 + t + 1], None,
                                op0=AT.is_equal)
                    chunk_cache[s][ci] = (ohen, wt)
                    return ohen, wt

                for j in range(NBLK):
                    n_ev = int(T[0][j] + T[1][j])
                    ps = ps_agg.tile([128, 130], F32, tag="ps", name="ps")
                    ev = 0
                    for s in range(2):
                        for t in range(int(T[s][j])):
                            gt = int(base_tile[s, j]) + t
                            ohen, wt = ensure_chunk(s, gt // G)
                            off = gt % G
                            nc.tensor.matmul(ps[:], ohen[:, off, :],
                                             wt[:, off, 0:130],
                                             start=(ev == 0),
                                             stop=(ev == n_ev - 1))
                            ev += 1
                    # ---- phase C for block j ----
                    R = 128 if j < NBLK - 1 else last_rows
                    rc = pc_sb.tile([128, H], F32, tag="rc", name="rc")
                    nc.vector.reciprocal(rc[:], ps[:, 128:130])
                    agg = pc_sb.tile([128, 128], FP16, tag="agg", name="agg")
                    with nc.allow_low_precision(reason="fp16 mlp"):
                        if R < 128:
                            nc.vector.memset(agg[:], 0.0)
                        nc.vector.tensor_tensor(
                            agg[0:R, :].rearrange("p (h d) -> p h d", h=H),
                            ps[0:R, 0:128].rearrange("p (h d) -> p h d", h=H),
                            rc[0:R, :].rearrange("p (h o) -> p h o", o=1)
                            .to_broadcast((R, H, D)),
                            op=AT.mult)
                    tp = pc_ps.tile([128, 128], F32, tag="cps", name="tp")
                    nc.tensor.matmul(tp[:], agg[:], ident[:], start=True,
                                     stop=True)
                    aggT = pc_sb.tile([128, 128], FP16, tag="aggT", name="aggT")
                    nc.scalar.copy(aggT[:], tp[:])
                    o1p = pc_ps.tile([128, 128], F32, tag="cps", name="o1p")
                    nc.tensor.matmul(o1p[:], outW[:], aggT[:], start=True,
                                     stop=True)
                    o1 = pc_sb.tile([128, 128], FP16, tag="o1", name="o1")
                    nc.scalar.activation(o1[:], o1p[:], ACTF.Relu,
                                         bias=outb[:, 0:1])
                    phpt = pc_ps.tile([128, 128], F32, tag="cps", name="phpt")
                    php = phpt[:, 0:P_OUT]
                    nc.tensor.matmul(php, o1[:], phW[:], start=True,
                                     stop=True)
                    z = pc_sb.tile([128, P_OUT], F32, tag="z", name="z")
                    nc.vector.tensor_tensor(z[:], php, phb[:], op=AT.add)
                    ez = pc_sb.tile([128, P_OUT], F32, tag="ez", name="ez")
                    nc.scalar.activation(ez[:], z[:], ACTF.Exp)
                    sm = pc_sb.tile([128, 1], F32, tag="sm", name="sm")
                    nc.vector.tensor_reduce(sm[:], ez[:],
                                            axis=mybir.AxisListType.X, op=AT.add)
                    rc2 = pc_sb.tile([128, 1], F32, tag="rc2", name="rc2")
                    nc.vector.reciprocal(rc2[:], sm[:])
                    ot = pc_sb.tile([128, P_OUT], F32, tag="ot", name="ot")
                    nc.vector.tensor_scalar(ot[:], ez[:], rc2[:, 0:1], None,
                                            op0=AT.mult)
                    nc.sync.dma_start(out_d[j * 128:j * 128 + R, :], ot[0:R, :])

    nc.compile()
    return nc


_CACHE = {}


def kernel(**inputs) -> np.ndarray:
    meta, in_maps = prep(**inputs)
    key = "nc"
    if key not in _CACHE:
        _CACHE[key] = build(meta)
    nc = _CACHE[key]
    res = run_bass_kernel_spmd(nc, in_maps, core_ids=list(range(N_CORES)))
    out = np.concatenate([res.results[c]["out"] for c in range(N_CORES)],
                         axis=0)
    return out.astype(np.float32)


# revision 10
# speedup vs baseline: 1.9550x; 1.0584x over previous
"""GATv2 actor layer (nn_GATv2Actor) on 8 TRN2 NeuronCores via Bass/Tile.

Self-contained: kernel(**inputs) takes the full (unsharded) inputs of
reference.setup_inputs() and returns the full [50000, 4] float32 output.

Distribution (edge-parallel by destination-node range): node n is owned by
core n // 6250; each core handles all edges whose destination lies in its
range (plus its self-loops), so the segment softmax and scatter-add are
core-local and output rows are disjoint (host concatenates).

Math restructure vs the straightforward edge pipeline:
  logits[e,h] = sum_d attn[h,d]*leaky(st[e,h,d])
              = sum_d sign(attn)[h,d] * leaky(|attn|[h,d]*st[e,h,d])
  (leaky_relu is positively homogeneous), so |attn| is folded into pair_W/
  pair_b on the host and the weighted reduce becomes a 2-column TensorE
  matmul against a constant +-1 sign mask.

Per-core phases:
  A: node tables. asv[n] = [|attn|-scaled h@pair_W_src | h@value_W] (fp16,
     512B rows, DRAM, gathered by edge src); adst[n] (dst half + bias,
     SBUF-resident per core).
  B: edges sorted by (dst, src), 128/partition-dim tiles, two streams by
     src<32768 (int16 gather indices). Per tile:
       - dma_gather asv rows (Pool desc-gen, 512B rows - no small-desc
         penalty on the shared DMA resource)
       - one-hot(n on partitions) oh_n_e built by DVE TensorTensor against
         a host-streamed partition-broadcast drel (plain packed operands
         keep the 2x DVE mode; AP broadcasts would drop to f32 rate)
       - st^T[d,e] in PSUM via two TensorE matmuls: identity-moving
         transpose of the gathered att half + adst_blk @ oh_n_e
       - leaky-relu on the Scalar engine (PSUM-in, 8-tile groups)
       - logits = matmul(lr_T, signmask) accumulated per-chunk in PSUM
       - exp on Scalar engine (fp16 copy into wt cols 128:130 + f32 copy
         for scaling), wt = v * ex via 4x-mode tensor_scalar per head
       - one-hot(e on partitions) oh_e_n via 4x-mode tensor_scalar
         is_equal(iota, drel) and a scatter matmul accumulating
         [ex-weighted values | ex] into the block's PSUM accumulator
  C: per 128-node block: agg = U/denom (fused divide), output MLP + phase
     softmax, DMA out.

SPMD: one program for all 8 cores; per-(stream,block) tile counts padded to
the max over cores; padded edges carry drel=-1 so their one-hot columns are
zero and they contribute nothing.
"""
import math
import sys

import numpy as np

sys.path.insert(0, "/opt/trn_rl_repo")

import concourse.bass as bass  # noqa: E402
import concourse.tile as tile  # noqa: E402
from concourse import bacc, mybir  # noqa: E402
from concourse.bass_utils import run_bass_kernel_spmd  # noqa: E402

FP16 = mybir.dt.float16
F32 = mybir.dt.float32
I16 = mybir.dt.int16
AT = mybir.AluOpType
ACTF = mybir.ActivationFunctionType

F = 128      # feature dim
H = 2        # heads
D = 64       # head dim
P_OUT = 4    # phases
N_CORES = 8


def prep(h_int, edge_index, pair_W, pair_b, attn_w, value_W, out_W, out_b,
         phase_W, phase_b, n_cores=N_CORES, G=24, split=32768, A_CH=8):
    """Host-side index preprocessing + input packing. Returns (meta, in_maps)."""
    h = np.asarray(h_int, np.float32)
    ei = np.asarray(edge_index)
    pair_W = np.asarray(pair_W, np.float32)
    pair_b = np.asarray(pair_b, np.float32)
    attn_w = np.asarray(attn_w, np.float32)
    value_W = np.asarray(value_W, np.float32)
    out_W = np.asarray(out_W, np.float32)
    out_b = np.asarray(out_b, np.float32)
    phase_W = np.asarray(phase_W, np.float32)
    phase_b = np.asarray(phase_b, np.float32)
    N = h.shape[0]
    assert N % n_cores == 0
    NPC = N // n_cores
    NBLK = (NPC + 127) // 128
    NPAD = ((N + 127) // 128) * 128
    assert NPAD - split < 32768 and split < 32768 + 1

    src = np.concatenate([ei[0], np.arange(N)]).astype(np.int64)
    dst = np.concatenate([ei[1], np.arange(N)]).astype(np.int64)
    core = dst // NPC

    percore = []
    counts = np.zeros((n_cores, 2, NBLK), np.int64)
    for c in range(n_cores):
        m = core == c
        es = src[m]
        ed = dst[m] - c * NPC
        o = np.lexsort((es, ed))
        es, ed = es[o], ed[o]
        lo = es < split
        percore.append((es, ed, lo))
        for si in range(2):
            msk = lo if si == 0 else ~lo
            counts[c, si] = np.bincount(ed[msk] // 128, minlength=NBLK)
    T = np.ceil(counts.max(axis=0) / 128.0).astype(np.int64)  # [2, NBLK]
    tiles = T.sum(axis=1)
    L = tiles * 128
    base_tile = np.zeros((2, NBLK + 1), np.int64)
    base_tile[:, 1:] = np.cumsum(T, axis=1)

    f16 = np.float16
    aw = np.abs(attn_w)                      # [H, D] magnitudes
    sg = np.sign(attn_w).astype(np.float32)  # [H, D] signs (+-1 or 0)
    # |attn|-folded weight halves
    Wsrc0 = pair_W[0, :F] * aw[0][None, :]
    Wsrc1 = pair_W[1, :F] * aw[1][None, :]
    Wdst0 = pair_W[0, F:] * aw[0][None, :]
    Wdst1 = pair_W[1, F:] * aw[1][None, :]
    W_asv = np.concatenate([Wsrc0, Wsrc1, value_W[0], value_W[1]],
                           axis=1).astype(f16)
    W_dst = np.concatenate([Wdst0, Wdst1], axis=1).astype(f16)
    bias_sc = (pair_b * aw).reshape(-1)      # [128] |attn|-scaled bias
    bias_bc = np.broadcast_to(bias_sc.astype(np.float32), (128, F)).copy()
    # sign mask [128, 2]: row d, col h = sign(attn[h, d-64h]) if d in head h
    signmask = np.zeros((128, H), np.float32)
    signmask[0:64, 0] = sg[0]
    signmask[64:128, 1] = sg[1]
    signmask = signmask.astype(f16)
    iota_bc = np.broadcast_to(np.arange(128, dtype=f16), (128, 128)).copy()
    iota_col = np.broadcast_to(np.arange(128, dtype=f16)[:, None],
                               (128, G * 128)).copy()
    ident16 = np.eye(128, dtype=f16)
    out_Wt = np.asarray(out_W, f16)
    out_b_c = np.asarray(out_b, np.float32).reshape(128, 1).copy()
    phase_Wt = np.asarray(phase_W, f16)
    phase_b_bc = np.broadcast_to(np.asarray(phase_b, np.float32),
                                 (128, P_OUT)).copy()

    hp = np.zeros((NPAD, F), np.float32)
    hp[:N] = h
    hT16 = np.ascontiguousarray(hp.T.astype(f16))

    shared = dict(hT16=hT16, W_asv=W_asv, W_dst=W_dst, bias_bc=bias_bc,
                  signmask=signmask, iota_bc=iota_bc, iota_col=iota_col,
                  ident16=ident16, out_Wt=out_Wt, out_b=out_b_c,
                  phase_Wt=phase_Wt, phase_b_bc=phase_b_bc)

    in_maps = []
    for c in range(n_cores):
        es, ed, lo = percore[c]
        m = {}
        for si in range(2):
            msk = lo if si == 0 else ~lo
            es_s, ed_s = es[msk], ed[msk]
            gidx = np.zeros(L[si], np.int16)
            drel = np.full(L[si], -1.0, np.float32)
            starts = np.searchsorted(ed_s, np.arange(NBLK + 1) * 128)
            for j in range(NBLK):
                seg = slice(starts[j], starts[j + 1])
                n = starts[j + 1] - starts[j]
                b0 = base_tile[si, j] * 128
                gidx[b0:b0 + n] = (es_s[seg] - (split if si else 0)).astype(np.int16)
                drel[b0:b0 + n] = (ed_s[seg] - j * 128).astype(np.float32)
            m[f"gw{si}"] = np.tile(gidx.reshape(-1, 16).T, (8, 1)).copy()
            m[f"drf{si}"] = np.ascontiguousarray(drel.reshape(-1, 128).T)
            m[f"drbc{si}"] = np.broadcast_to(
                drel.astype(f16)[None, :], (128, int(L[si]))).copy()
        hl = np.zeros((NBLK * 128, F), np.float32)
        hl[:NPC] = h[c * NPC:(c + 1) * NPC]
        m["hlocT16"] = np.ascontiguousarray(hl.T.astype(f16))
        m.update(shared)
        in_maps.append(m)

    meta = dict(N=N, NPC=NPC, NBLK=NBLK, NPAD=NPAD, split=split, G=G,
                A_CH=A_CH, T=T, tiles=tiles, L=L, base_tile=base_tile,
                n_cores=n_cores)
    return meta, in_maps


def build(meta):
    NPC, NBLK, NPAD = meta["NPC"], meta["NBLK"], meta["NPAD"]
    split, G, A_CH = meta["split"], meta["G"], meta["A_CH"]
    T, tiles, L = meta["T"], meta["tiles"], meta["L"]
    base_tile = meta["base_tile"]
    last_rows = NPC - (NBLK - 1) * 128

    # tile -> block id per stream (static schedule)
    tile_block = [[], []]
    for s in range(2):
        for j in range(NBLK):
            tile_block[s].extend([j] * int(T[s][j]))

    nc = bacc.Bacc(None, target_bir_lowering=False, debug=False)

    hT_d = nc.dram_tensor("hT16", [128, NPAD], FP16, kind="ExternalInput")
    hloc_d = nc.dram_tensor("hlocT16", [128, NBLK * 128], FP16,
                            kind="ExternalInput")
    gw_d = [nc.dram_tensor(f"gw{s}", [128, int(L[s]) // 16], I16,
                           kind="ExternalInput") for s in range(2)]
    drf_d = [nc.dram_tensor(f"drf{s}", [128, int(tiles[s])], F32,
                            kind="ExternalInput") for s in range(2)]
    drbc_d = [nc.dram_tensor(f"drbc{s}", [128, int(L[s])], FP16,
                             kind="ExternalInput") for s in range(2)]
    Wasv_d = nc.dram_tensor("W_asv", [128, 256], FP16, kind="ExternalInput")
    Wdst_d = nc.dram_tensor("W_dst", [128, 128], FP16, kind="ExternalInput")
    bias_d = nc.dram_tensor("bias_bc", [128, 128], F32, kind="ExternalInput")
    sgn_d = nc.dram_tensor("signmask", [128, H], FP16, kind="ExternalInput")
    iota_d = nc.dram_tensor("iota_bc", [128, 128], FP16, kind="ExternalInput")
    iotac_d = nc.dram_tensor("iota_col", [128, G * 128], FP16,
                             kind="ExternalInput")
    ident_d = nc.dram_tensor("ident16", [128, 128], FP16, kind="ExternalInput")
    outW_d = nc.dram_tensor("out_Wt", [128, 128], FP16, kind="ExternalInput")
    outb_d = nc.dram_tensor("out_b", [128, 1], F32, kind="ExternalInput")
    phW_d = nc.dram_tensor("phase_Wt", [128, P_OUT], FP16, kind="ExternalInput")
    phb_d = nc.dram_tensor("phase_b_bc", [128, P_OUT], F32, kind="ExternalInput")

    asv_d = nc.dram_tensor("asv_tab", [NPAD, 256], FP16)
    out_d = nc.dram_tensor("out", [NPC, P_OUT], F32, kind="ExternalOutput")

    with tile.TileContext(nc) as tc:
        with tc.tile_pool(name="consts", bufs=1) as pc:
            def cload(name, dram, shape, dtype):
                t = pc.tile(shape, dtype, tag=name, name=name)
                nc.sync.dma_start(t[:], dram[:])
                return t
            Wasv = cload("Wasv", Wasv_d, [128, 256], FP16)
            Wdst = cload("Wdst", Wdst_d, [128, 128], FP16)
            bias = cload("bias", bias_d, [128, 128], F32)
            sgn = cload("sgn", sgn_d, [128, H], FP16)
            iota = cload("iota", iota_d, [128, 128], FP16)
            iotac = cload("iotac", iotac_d, [128, G * 128], FP16)
            ident = cload("ident", ident_d, [128, 128], FP16)
            outW = cload("outW", outW_d, [128, 128], FP16)
            outb = cload("outb", outb_d, [128, 1], F32)
            phW = cload("phW", phW_d, [128, P_OUT], FP16)
            phb = cload("phb", phb_d, [128, P_OUT], F32)
            gw = [cload(f"gw{s}", gw_d[s], [128, int(L[s]) // 16], I16)
                  for s in range(2)]
            drf = [cload(f"drf{s}", drf_d[s], [128, int(tiles[s])], F32)
                   for s in range(2)]
            # adst table: SBUF-resident per core [128, NBLK, 128] fp16
            adst_res = pc.tile([128, NBLK, 128], FP16, tag="adst_res",
                               name="adst_res")

            # ---------------- phase A: node tables ----------------
            asv_re = asv_d[:].rearrange("(i p) f -> p i f", p=128)
            A_TILES = NPAD // 128
            with tc.tile_pool(name="pa_in", bufs=3) as pa_in, \
                 tc.tile_pool(name="pa_ps", bufs=2, space="PSUM") as pa_ps, \
                 tc.tile_pool(name="pa_out", bufs=3) as pa_out:
                copy_flip = 0
                for t0 in range(0, A_TILES, A_CH):
                    ac = min(A_CH, A_TILES - t0)
                    hc = pa_in.tile([128, A_CH, 128], FP16, tag="hc", name="hc")
                    nc.sync.dma_start(hc[:, 0:ac, :],
                                      hT_d[:, t0 * 128:(t0 + ac) * 128]
                                      .rearrange("p (i n) -> p i n", i=ac))
                    mm = pa_ps.tile([128, A_CH, 256], F32, tag="mm", name="mm")
                    for i in range(ac):
                        nc.tensor.matmul(mm[:, i, :], hc[:, i, :], Wasv[:],
                                         start=True, stop=True)
                    ao = pa_out.tile([128, A_CH, 256], FP16, tag="ao", name="ao")
                    r = copy_flip % 3
                    copy_flip += 1
                    if r == 1:
                        nc.vector.tensor_copy(ao[:, 0:ac, :], mm[:, 0:ac, :])
                    else:
                        nc.scalar.copy(ao[:, 0:ac, :], mm[:, 0:ac, :])
                    nc.sync.dma_start(asv_re[:, t0:t0 + ac, :], ao[:, 0:ac, :])
                for t0 in range(0, NBLK, A_CH):
                    ac = min(A_CH, NBLK - t0)
                    hc = pa_in.tile([128, A_CH, 128], FP16, tag="hc", name="hc2")
                    nc.sync.dma_start(hc[:, 0:ac, :],
                                      hloc_d[:, t0 * 128:(t0 + ac) * 128]
                                      .rearrange("p (i n) -> p i n", i=ac))
                    mm = pa_ps.tile([128, A_CH, 256], F32, tag="mm", name="mm2")
                    for i in range(ac):
                        nc.tensor.matmul(mm[:, i, 0:128], hc[:, i, :], Wdst[:],
                                         start=True, stop=True)
                    with nc.allow_low_precision(reason="fp16 edge tables"):
                        nc.vector.tensor_tensor(
                            adst_res[:, t0:t0 + ac, :], mm[:, 0:ac, 0:128],
                            bias[:].rearrange("p (o n) -> p o n", o=1)
                            .to_broadcast((128, ac, 128)),
                            op=AT.add)

            tc.strict_bb_all_engine_barrier()

            # ---------------- phase B + C ----------------
            asv_base = [asv_d[0:split, :], asv_d[split:NPAD, :]]
            chunk_cache = [dict(), dict()]

            with tc.tile_pool(name="pg_asv", bufs=2) as pg_asv, \
                 tc.tile_pool(name="pg_dr", bufs=2) as pg_dr, \
                 tc.tile_pool(name="pb_ohne", bufs=2) as pb_ohne, \
                 tc.tile_pool(name="pb_ohen", bufs=2) as pb_ohen, \
                 tc.tile_pool(name="pb_lr", bufs=2) as pb_lr, \
                 tc.tile_pool(name="pb_wt", bufs=2) as pb_wt, \
                 tc.tile_pool(name="ps_st", bufs=2, space="PSUM") as ps_st, \
                 tc.tile_pool(name="ps_lg", bufs=1, space="PSUM") as ps_lg, \
                 tc.tile_pool(name="ps_agg", bufs=2, space="PSUM") as ps_agg, \
                 tc.tile_pool(name="pc_ps", bufs=2, space="PSUM") as pc_ps, \
                 tc.tile_pool(name="pc_sb", bufs=2) as pc_sb:

                def ensure_chunk(s, ci):
                    if ci in chunk_cache[s]:
                        return chunk_cache[s][ci]
                    t0 = ci * G
                    g = min(G, int(tiles[s]) - t0)
                    GSUB = 8  # <=1024 idxs/dma_gather: 64 descs per SDMA
                    # engine, safely under the 128-slot DGE ring
                    asv_g = pg_asv.tile([128, G, 256], FP16, tag="asv_g",
                                        name="asv_g")
                    for k in range(0, g, GSUB):
                        gs = min(GSUB, g - k)
                        ne = gs * 128
                        nc.gpsimd.dma_gather(
                            asv_g[:, k:k + gs, :], asv_base[s],
                            gw[s][:, (t0 + k) * 8:(t0 + k + gs) * 8],
                            ne, ne, 256)
                    drbc = pg_dr.tile([128, G, 128], FP16, tag="drbc",
                                      name="drbc")
                    nc.sync.dma_start(
                        drbc[:, 0:g, :],
                        drbc_d[s][:, t0 * 128:(t0 + g) * 128]
                        .rearrange("p (t e) -> p t e", t=g))
                    # one-hot with n on partitions (for a_dst matmul)
                    ohne = pb_ohne.tile([128, G, 128], FP16, tag="ohne",
                                        name="ohne")
                    nc.vector.tensor_tensor(
                        ohne[:, 0:g, :], drbc[:, 0:g, :],
                        iotac[:].rearrange("p (t e) -> p t e", t=G)[:, 0:g, :],
                        op=AT.is_equal)
                    # st^T per tile into PSUM (8-tile bank groups)
                    lr = pb_lr.tile([128, G, 128], FP16, tag="lr", name="lr")
                    lgp = ps_lg.tile([128, G, H], F32, tag="lgp", name="lgp")
                    for k in range(0, g, 4):
                        gs = min(4, g - k)
                        stp = ps_st.tile([128, 4, 128], F32, tag="stp",
                                         name="stp")
                        for i in range(gs):
                            t = k + i
                            j = tile_block[s][t0 + t]
                            nc.tensor.matmul(stp[:, i, :],
                                             asv_g[:, t, 0:128], ident[:],
                                             start=True, stop=False)
                            nc.tensor.matmul(stp[:, i, :],
                                             adst_res[:, j, :],
                                             ohne[:, t, :],
                                             start=False, stop=True)
                        nc.scalar.activation(lr[:, k:k + gs, :],
                                             stp[:, 0:gs, :], ACTF.Prelu,
                                             alpha=0.2)
                        for i in range(gs):
                            t = k + i
                            nc.tensor.matmul(lgp[:, t, :], lr[:, t, :],
                                             sgn[:], start=True, stop=True)
                    # exp -> fp16 into wt cols 128:130 and f32 scratch
                    wt = pb_wt.tile([128, G, 130], FP16, tag="wt", name="wt")
                    nc.scalar.activation(wt[:, 0:g, 128:130], lgp[:, 0:g, :],
                                         ACTF.Exp)
                    # wt = v * ex (per tile per head, 4x tensor_scalar) and
                    # one-hot with e on partitions (for scatter matmul)
                    ohen = pb_ohen.tile([128, G, 128], FP16, tag="ohen",
                                        name="ohen")
                    with nc.allow_low_precision(reason="fp16 edge math"):
                        nc.gpsimd.tensor_tensor(
                            wt[:, 0:g, 0:128].rearrange(
                                "p g (h d) -> p g h d", h=H),
                            asv_g[:, 0:g, 128:256].rearrange(
                                "p g (h d) -> p g h d", h=H),
                            wt[:, 0:g, 128:130].rearrange(
                                "p g (h o) -> p g h o", o=1)
                            .to_broadcast((128, g, H, D)),
                            op=AT.mult)
                        for t in range(g):
                            nc.vector.tensor_scalar(
                                ohen[:, t, :], iota[:],
                                drf[s][:, t0 + t:t0 + t + 1], None,
                                op0=AT.is_equal)
                    chunk_cache[s][ci] = (ohen, wt)
                    return ohen, wt

                for j in range(NBLK):
                    n_ev = int(T[0][j] + T[1][j])
                    ps = ps_agg.tile([128, 130], F32, tag="ps", name="ps")
                    ev = 0
                    for s in range(2):
                        for t in range(int(T[s][j])):
                            gt = int(base_tile[s, j]) + t
                            ohen, wt = ensure_chunk(s, gt // G)
                            off = gt % G
                            nc.tensor.matmul(ps[:], ohen[:, off, :],
                                             wt[:, off, 0:130],
                                             start=(ev == 0),
                                             stop=(ev == n_ev - 1))
                            ev += 1
                    # ---- phase C for block j ----
                    R = 128 if j < NBLK - 1 else last_rows
                    rc = pc_sb.tile([128, H], F32, tag="rc", name="rc")
                    nc.vector.reciprocal(rc[:], ps[:, 128:130])
                    agg = pc_sb.tile([128, 128], FP16, tag="agg", name="agg")
                    with nc.allow_low_precision(reason="fp16 mlp"):
                        if R < 128:
                            nc.vector.memset(agg[:], 0.0)
                        nc.vector.tensor_tensor(
                            agg[0:R, :].rearrange("p (h d) -> p h d", h=H),
                            ps[0:R, 0:128].rearrange("p (h d) -> p h d", h=H),
                            rc[0:R, :].rearrange("p (h o) -> p h o", o=1)
                            .to_broadcast((R, H, D)),
                            op=AT.mult)
                    tp = pc_ps.tile([128, 128], F32, tag="cps", name="tp")
                    nc.tensor.matmul(tp[:], agg[:], ident[:], start=True,
                                     stop=True)
                    aggT = pc_sb.tile([128, 128], FP16, tag="aggT", name="aggT")
                    nc.scalar.copy(aggT[:], tp[:])
                    o1p = pc_ps.tile([128, 128], F32, tag="cps", name="o1p")
                    nc.tensor.matmul(o1p[:], outW[:], aggT[:], start=True,
                                     stop=True)
                    o1 = pc_sb.tile([128, 128], FP16, tag="o1", name="o1")
                    nc.scalar.activation(o1[:], o1p[:], ACTF.Relu,
                                         bias=outb[:, 0:1])
                    phpt = pc_ps.tile([128, 128], F32, tag="cps", name="phpt")
                    php = phpt[:, 0:P_OUT]
                    nc.tensor.matmul(php, o1[:], phW[:], start=True,
                                     stop=True)
                    z = pc_sb.tile([128, P_OUT], F32, tag="z", name="z")
                    nc.vector.tensor_tensor(z[:], php, phb[:], op=AT.add)
                    ez = pc_sb.tile([128, P_OUT], F32, tag="ez", name="ez")
                    nc.scalar.activation(ez[:], z[:], ACTF.Exp)
                    sm = pc_sb.tile([128, 1], F32, tag="sm", name="sm")
                    nc.vector.tensor_reduce(sm[:], ez[:],
                                            axis=mybir.AxisListType.X, op=AT.add)
                    rc2 = pc_sb.tile([128, 1], F32, tag="rc2", name="rc2")
                    nc.vector.reciprocal(rc2[:], sm[:])
                    ot = pc_sb.tile([128, P_OUT], F32, tag="ot", name="ot")
                    nc.vector.tensor_scalar(ot[:], ez[:], rc2[:, 0:1], None,
                                            op0=AT.mult)
                    nc.sync.dma_start(out_d[j * 128:j * 128 + R, :], ot[0:R, :])

    nc.compile()
    return nc


_CACHE = {}


def kernel(**inputs) -> np.ndarray:
    meta, in_maps = prep(**inputs)
    key = "nc"
    if key not in _CACHE:
        _CACHE[key] = build(meta)
    nc = _CACHE[key]
    res = run_bass_kernel_spmd(nc, in_maps, core_ids=list(range(N_CORES)))
    out = np.concatenate([res.results[c]["out"] for c in range(N_CORES)],
                         axis=0)
    return out.astype(np.float32)
